# revision 2
# baseline (speedup 1.0000x reference)
import sys

if "/opt/trn_rl_repo" not in sys.path:
    sys.path.insert(0, "/opt/trn_rl_repo")

import numpy as np
import ml_dtypes

import concourse.bass as bass
from concourse import bacc
import concourse.mybir as mybir
from concourse.tile import TileContext
from concourse.bass_utils import run_bass_kernel_spmd
from concourse.masks import make_identity

B, E, DIM, RANK, KTOP, P, H, W, FREQ = 16, 4, 64, 32, 2, 8, 128, 128, 64
NOISE_STD = 1.0 / E
NCORES = 8
SPC = B // NCORES  # samples per core
ROWS = 16          # block rows (16 blocks? no: H/ROWS = 8 blocks per sample)
NBLK = H // ROWS
HALO = 3
PADW = W + 2 * HALO   # 134
PADR = ROWS + 2 * HALO  # 22

f32 = mybir.dt.float32
bf16 = mybir.dt.bfloat16
AF = mybir.ActivationFunctionType
ALU = mybir.AluOpType
BF = ml_dtypes.bfloat16

_CACHE = {}


def _host_gates(x, freq_emb, noise, gate_w, freq_gate_w):
    pooled = x.reshape(B, DIM, H * W).mean(axis=2)
    logits = pooled @ gate_w.T + freq_emb @ freq_gate_w.T
    noisy = logits + noise * NOISE_STD
    m = noisy.max(axis=-1, keepdims=True)
    ex = np.exp(noisy - m)
    scores = ex / ex.sum(axis=-1, keepdims=True)
    gates = np.zeros_like(scores)
    # top-k, ties broken by lower index (matches jax.lax.top_k)
    order = np.argsort(-scores, axis=-1, kind="stable")
    for b in range(B):
        for k in range(KTOP):
            e = order[b, k]
            gates[b, e] = scores[b, e]
    return gates


def _fft_mats():
    m_fwd = np.zeros((64, 80), np.float64)
    for p in range(64):
        img = np.zeros((P, P))
        img.flat[p] = 1.0
        F = np.fft.rfft2(img)  # [8,5] complex
        m_fwd[p, :40] = F.real.ravel()
        m_fwd[p, 40:] = F.imag.ravel()
    m_inv = np.zeros((80, 64), np.float64)
    for f in range(40):
        Z = np.zeros((P, P // 2 + 1), np.complex128)
        Z.flat[f] = 1.0
        m_inv[f] = np.fft.irfft2(Z, s=(P, P)).ravel()
        Z.flat[f] = 1j
        m_inv[40 + f] = np.fft.irfft2(Z, s=(P, P)).ravel()
    return m_fwd.astype(np.float32), m_inv.astype(np.float32)


def _build():
    nc = bacc.Bacc()
    dp = nc.declare_dram_parameter
    xp = dp("x", [SPC, DIM, H, W], f32, isOutput=False)
    xbp = dp("x_bf", [SPC, DIM, H, W], bf16, isOutput=False)
    sp = dp("shared", [SPC, DIM, H, W], bf16, isOutput=False)
    w_xr = dp("w_xr", [DIM, 128], bf16, isOutput=False)
    w_q = dp("w_q", [128, 128], bf16, isOutput=False)
    w_k = dp("w_k", [128, 128], bf16, isOutput=False)
    w_v = dp("w_v", [128, 128], bf16, isOutput=False)
    t_q = dp("t_q", [128, 9], f32, isOutput=False)
    t_k = dp("t_k", [128, 49], f32, isOutput=False)
    t_v = dp("t_v", [128, 49], f32, isOutput=False)
    w_p1 = dp("w_p1", [DIM, 128], bf16, isOutput=False)
    w_po = dp("w_po", [128, 128], bf16, isOutput=False)
    po_b = dp("po_b", [128, 1], f32, isOutput=False)
    ln_w = dp("ln_w", [128, 1], f32, isOutput=False)
    ln_b = dp("ln_b", [128, 1], f32, isOutput=False)
    m_fr = dp("m_fr", [64, 40], bf16, isOutput=False)
    m_fi = dp("m_fi", [64, 40], bf16, isOutput=False)
    m_ir = dp("m_ir", [40, 64], bf16, isOutput=False)
    m_ii = dp("m_ii", [40, 64], bf16, isOutput=False)
    ones_blk = dp("ones_blk", [128, 4], bf16, isOutput=False)
    ind = dp("ind", [4, 128], bf16, isOutput=False)
    eps_ap = dp("eps_ap", [4, 1], f32, isOutput=False)
    w_comb = dp("w_comb", [SPC, 128, DIM], bf16, isOutput=False)
    resid = dp("resid", [SPC, DIM, 1], f32, isOutput=False)
    outp = dp("out", [SPC, DIM, H, W], f32, isOutput=True)

    RB2, RR, WB, WW = 2, 8, 16, 8

    def RM(t):  # row-major memory, patch-major iteration
        return t[...].rearrange("p (rb rr wb ww) -> p rb wb rr ww",
                                rb=RB2, rr=RR, wb=WB, ww=WW)

    def PM(t):  # patch-major memory, patch-major iteration
        return t[...].rearrange("p (rb wb rr ww) -> p rb wb rr ww",
                                rb=RB2, wb=WB, rr=RR, ww=WW)

    with TileContext(nc) as tc:
        with (
            tc.tile_pool(name="const", bufs=1) as cpool,
            tc.tile_pool(name="io", bufs=2) as iop,
            tc.tile_pool(name="work", bufs=1) as wk,
            tc.tile_pool(name="ps", bufs=2, space="PSUM") as ps,
            tc.tile_pool(name="psb", bufs=4, space="PSUM") as psb,
        ):
            idt = cpool.tile([128, 128], bf16, tag="idt")
            make_identity(nc, idt[:, :])
            cw = {}
            for name, hnd, shp, dt in [
                ("w_xr", w_xr, [DIM, 128], bf16),
                ("w_q", w_q, [128, 128], bf16),
                ("w_k", w_k, [128, 128], bf16),
                ("w_v", w_v, [128, 128], bf16),
                ("t_q", t_q, [128, 9], f32),
                ("t_k", t_k, [128, 49], f32),
                ("t_v", t_v, [128, 49], f32),
                ("w_p1", w_p1, [DIM, 128], bf16),
                ("w_po", w_po, [128, 128], bf16),
                ("po_b", po_b, [128, 1], f32),
                ("ln_w", ln_w, [128, 1], f32),
                ("ln_b", ln_b, [128, 1], f32),
                ("m_fr", m_fr, [64, 40], bf16),
                ("m_fi", m_fi, [64, 40], bf16),
                ("m_ir", m_ir, [40, 64], bf16),
                ("m_ii", m_ii, [40, 64], bf16),
                ("ones_blk", ones_blk, [128, 4], bf16),
                ("ind", ind, [4, 128], bf16),
                ("eps_ap", eps_ap, [4, 1], f32),
            ]:
                t = cpool.tile(shp, dt, tag=name)
                nc.gpsimd.dma_start(out=t[...], in_=hnd[...])
                cw[name] = t

            NPIX = ROWS * W  # 2048
            for b in range(SPC):
                wcb = cpool.tile([128, DIM], bf16, tag="wcb")
                nc.gpsimd.dma_start(out=wcb[...], in_=w_comb[b])
                rsd = cpool.tile([DIM, 1], f32, tag="rsd")
                nc.gpsimd.dma_start(out=rsd[...], in_=resid[b])

                for blk in range(NBLK):
                    r0 = blk * ROWS
                    rlo = max(0, r0 - HALO)
                    rhi = min(H, r0 + ROWS + HALO)
                    nv = rhi - rlo
                    off = rlo - (r0 - HALO)

                    xt = iop.tile([DIM, PADR * W], bf16, tag="xt")
                    xt3 = xt[...].rearrange("p (r w) -> p r w", r=PADR, w=W)
                    nc.gpsimd.dma_start(out=xt3[:, :nv, :], in_=xbp[b, :, rlo:rhi, :])
                    sht = iop.tile([DIM, NPIX], bf16, tag="sht")
                    nc.gpsimd.dma_start(
                        out=sht[...].rearrange("p (r w) -> p r w", r=ROWS, w=W),
                        in_=sp[b, :, r0:r0 + ROWS, :])

                    xrf = wk.tile([128, PADR * W], bf16, tag="xrf")
                    qpre = wk.tile([128, PADR, PADW], bf16, tag="qpre")
                    kpre = wk.tile([128, PADR, PADW], bf16, tag="kpre")
                    vpre = wk.tile([128, PADR, PADW], bf16, tag="vpre")
                    for t in (qpre, kpre, vpre):
                        nc.gpsimd.memset(t[...], 0.0)

                    # xr = p0-stacked conv1x1(x), rows rlo..rhi (flat row-major)
                    a = 0
                    while a < nv:
                        nt = min(4, nv - a)
                        pt0 = ps.tile([128, 512], f32, tag="pp")
                        nc.tensor.matmul(pt0[:, :nt * W], cw["w_xr"][...],
                                         xt[:, a * W:(a + nt) * W],
                                         start=True, stop=True)
                        nc.vector.tensor_copy(out=xrf[:, (off + a) * W:(off + a + nt) * W],
                                              in_=pt0[:, :nt * W])
                        a += nt

                    # q/k/v pre = blockdiag conv1x1(xr) into padded bufs
                    for wname, dst in (("w_q", qpre), ("w_k", kpre), ("w_v", vpre)):
                        a = 0
                        while a < nv:
                            nt = min(4, nv - a)
                            pt0 = ps.tile([128, 512], f32, tag="pp")
                            nc.tensor.matmul(pt0[:, :nt * W], cw[wname][...],
                                             xrf[:, (off + a) * W:(off + a + nt) * W],
                                             start=True, stop=True)
                            nc.vector.tensor_copy(
                                out=dst[:, off + a:off + a + nt, HALO:HALO + W],
                                in_=pt0[:, :nt * W].rearrange(
                                    "p (r w) -> p r w", r=nt, w=W))
                            a += nt

                    # depthwise convs (shifted MACs on DVE), row-major out
                    qd = wk.tile([128, NPIX], bf16, tag="qd")
                    kd = wk.tile([128, NPIX], bf16, tag="kd")
                    vd = wk.tile([128, NPIX], bf16, tag="vd")
                    for pre, dst, taps, rad in ((qpre, qd, cw["t_q"], 1),
                                                (kpre, kd, cw["t_k"], 3),
                                                (vpre, vd, cw["t_v"], 3)):
                        ntap = 2 * rad + 1
                        d3 = dst[...].rearrange("p (r w) -> p r w", r=ROWS, w=W)
                        first = True
                        for di in range(ntap):
                            for dj in range(ntap):
                                sl = pre[:, HALO - rad + di:HALO - rad + di + ROWS,
                                         HALO - rad + dj:HALO - rad + dj + W]
                                tap = taps[:, di * ntap + dj:di * ntap + dj + 1]
                                if first:
                                    nc.vector.tensor_scalar_mul(d3, sl, tap)
                                    first = False
                                else:
                                    nc.vector.scalar_tensor_tensor(
                                        out=d3, in0=sl, scalar=tap,
                                        in1=d3, op0=ALU.mult, op1=ALU.add)

                    # reorder q,k to patch-major for transposes
                    qdp = wk.tile([128, NPIX], bf16, tag="qdp")
                    kdp = wk.tile([128, NPIX], bf16, tag="kdp")
                    for rb_ in range(RB2):
                        nc.vector.tensor_copy(out=PM(qdp)[:, rb_], in_=RM(qd)[:, rb_])
                        nc.vector.tensor_copy(out=PM(kdp)[:, rb_], in_=RM(kd)[:, rb_])

                    npat = (ROWS // P) * (W // P)  # 32
                    att = wk.tile([128, NPIX], bf16, tag="qd")
                    for ch in range(2):
                        pbase = ch * 16
                        qT = wk.tile([64, 2048], bf16, tag="qT")
                        kT = wk.tile([64, 2048], bf16, tag="kT")
                        for src_, dstT in ((qdp, qT), (kdp, kT)):
                            for g4 in range(4):
                                pt0 = psb.tile([128, 512], bf16, tag="pt")
                                for j in range(4):
                                    pi = (pbase + g4 * 4 + j) * 64
                                    nc.tensor.transpose(
                                        pt0[0:64, j * 128:(j + 1) * 128],
                                        src_[:, pi:pi + 64], idt[:, :])
                                nc.vector.tensor_copy(
                                    out=dstT[:, g4 * 512:(g4 + 1) * 512],
                                    in_=pt0[0:64, :])
                        qfr = wk.tile([40, 2048], bf16, tag="qfr")
                        qfi = wk.tile([40, 2048], bf16, tag="qfi")
                        kfr = wk.tile([40, 2048], bf16, tag="kfr")
                        kfi = wk.tile([40, 2048], bf16, tag="kfi")
                        for srcT, dre, dim_ in ((qT, qfr, qfi), (kT, kfr, kfi)):
                            for t in range(4):
                                for mf, dst_ in (("m_fr", dre), ("m_fi", dim_)):
                                    pt0 = ps.tile([128, 512], f32, tag="pp")
                                    nc.tensor.matmul(
                                        pt0[0:40, :], cw[mf][...],
                                        srcT[:, t * 512:(t + 1) * 512],
                                        start=True, stop=True)
                                    nc.vector.tensor_copy(
                                        out=dst_[:, t * 512:(t + 1) * 512],
                                        in_=pt0[0:40, :])
                        pror = wk.tile([40, 2048], bf16, tag="pror")
                        proi = wk.tile([40, 2048], bf16, tag="proi")
                        tmp = wk.tile([40, 2048], bf16, tag="tmp")
                        nc.vector.tensor_mul(pror[...], qfr[...], kfr[...])
                        nc.vector.tensor_mul(tmp[...], qfi[...], kfi[...])
                        nc.vector.tensor_sub(pror[...], pror[...], tmp[...])
                        nc.vector.tensor_mul(proi[...], qfr[...], kfi[...])
                        nc.vector.tensor_mul(tmp[...], qfi[...], kfr[...])
                        nc.vector.tensor_add(proi[...], proi[...], tmp[...])
                        oT = wk.tile([64, 2048], bf16, tag="oT")
                        for t in range(4):
                            pt0 = ps.tile([128, 512], f32, tag="pp")
                            nc.tensor.matmul(pt0[0:64, :], cw["m_ir"][...],
                                             pror[:, t * 512:(t + 1) * 512],
                                             start=True, stop=False)
                            nc.tensor.matmul(pt0[0:64, :], cw["m_ii"][...],
                                             proi[:, t * 512:(t + 1) * 512],
                                             start=False, stop=True)
                            nc.vector.tensor_copy(out=oT[:, t * 512:(t + 1) * 512],
                                                  in_=pt0[0:64, :])
                        for g8 in range(2):
                            pt0 = psb.tile([128, 512], bf16, tag="pt")
                            for j in range(8):
                                pi = g8 * 8 + j
                                nc.tensor.transpose(
                                    pt0[:, j * 64:(j + 1) * 64],
                                    oT[:, pi * 128:(pi + 1) * 128],
                                    idt[0:64, 0:64])
                            nc.vector.tensor_copy(
                                out=att[:, ch * 1024 + g8 * 512:ch * 1024 + (g8 + 1) * 512],
                                in_=pt0[:, :])

                    # group layernorm over 32-chan expert groups (patch-major)
                    sq = wk.tile([128, NPIX], bf16, tag="oT")
                    nc.scalar.activation(out=sq[...], in_=att[...], func=AF.Square)
                    st = wk.tile([4, 3 * NPIX], f32, tag="st")
                    for si, srct in ((0, att), (NPIX, sq)):
                        for t in range(NPIX // 512):
                            pt0 = ps.tile([128, 512], f32, tag="pp")
                            nc.tensor.matmul(pt0[0:4, :], cw["ones_blk"][...],
                                             srct[:, t * 512:(t + 1) * 512],
                                             start=True, stop=True)
                            nc.vector.tensor_copy(
                                out=st[:, si + t * 512:si + (t + 1) * 512],
                                in_=pt0[0:4, :])
                    MU, MS, T3 = 0, NPIX, 2 * NPIX
                    nc.vector.tensor_mul(st[:, T3:T3 + NPIX], st[:, MU:MU + NPIX],
                                         st[:, MU:MU + NPIX])
                    nc.vector.tensor_sub(st[:, MS:MS + NPIX], st[:, MS:MS + NPIX],
                                         st[:, T3:T3 + NPIX])
                    nc.scalar.activation(out=st[:, MS:MS + NPIX],
                                         in_=st[:, MS:MS + NPIX],
                                         func=AF.Sqrt, bias=cw["eps_ap"][...])
                    nc.vector.reciprocal(st[:, T3:T3 + NPIX], st[:, MS:MS + NPIX])
                    nc.vector.tensor_mul(st[:, MU:MU + NPIX], st[:, MU:MU + NPIX],
                                         st[:, T3:T3 + NPIX])
                    rb2a = wk.tile([4, NPIX], bf16, tag="rb2a")
                    rb2b = wk.tile([4, NPIX], bf16, tag="rb2b")
                    nc.vector.tensor_copy(out=rb2a[...], in_=st[:, T3:T3 + NPIX])
                    nc.vector.tensor_copy(out=rb2b[...], in_=st[:, MU:MU + NPIX])
                    Rb = wk.tile([128, NPIX], bf16, tag="Rb")
                    M2b = wk.tile([128, NPIX], bf16, tag="M2b")
                    for srcb, dstb in ((rb2a, Rb), (rb2b, M2b)):
                        for t in range(NPIX // 512):
                            pt0 = ps.tile([128, 512], f32, tag="pp")
                            nc.tensor.matmul(pt0[:, :], cw["ind"][...],
                                             srcb[:, t * 512:(t + 1) * 512],
                                             start=True, stop=True)
                            nc.vector.tensor_copy(out=dstb[:, t * 512:(t + 1) * 512],
                                                  in_=pt0[:, :])
                    attn = wk.tile([128, NPIX], bf16, tag="attn")
                    nc.vector.tensor_mul(attn[...], att[...], Rb[...])
                    nc.vector.tensor_sub(attn[...], attn[...], M2b[...])
                    nc.scalar.activation(out=attn[...], in_=attn[...],
                                         func=AF.Identity,
                                         bias=cw["ln_b"][...], scale=cw["ln_w"][...])
                    attv = wk.tile([128, NPIX], bf16, tag="kdp")
                    for rb_ in range(RB2):
                        nc.vector.tensor_mul(PM(attv)[:, rb_], PM(attn)[:, rb_], RM(vd)[:, rb_])

                    po = wk.tile([128, NPIX], bf16, tag="qdp")
                    for t in range(NPIX // 512):
                        pt0 = ps.tile([128, 512], f32, tag="pp")
                        nc.tensor.matmul(pt0[:, :], cw["w_po"][...],
                                         attv[:, t * 512:(t + 1) * 512],
                                         start=True, stop=True)
                        nc.scalar.activation(out=po[:, t * 512:(t + 1) * 512],
                                             in_=pt0[:, :], func=AF.Identity,
                                             bias=cw["po_b"][...])

                    gt = wk.tile([128, NPIX], bf16, tag="gt")
                    for t in range(NPIX // 512):
                        pt0 = ps.tile([128, 512], f32, tag="pp")
                        nc.tensor.matmul(pt0[:, :], cw["w_p1"][...],
                                         sht[:, t * 512:(t + 1) * 512],
                                         start=True, stop=True)
                        nc.scalar.activation(out=gt[:, t * 512:(t + 1) * 512],
                                             in_=pt0[:, :], func=AF.Silu)

                    attg = wk.tile([128, NPIX], bf16, tag="attg")
                    for rb_ in range(RB2):
                        nc.vector.tensor_mul(PM(attg)[:, rb_], PM(po)[:, rb_], RM(gt)[:, rb_])

                    ot = iop.tile([DIM, NPIX], f32, tag="ot")
                    ot_pm = wk.tile([DIM, NPIX], f32, tag="ot_pm")
                    xpm = wk.tile([DIM, NPIX], bf16, tag="xpm")
                    ioff = r0 - rlo
                    xr_view = xt[...][:, ioff * W:(ioff + ROWS) * W].rearrange(
                        "p (rb rr wb ww) -> p rb wb rr ww", rb=RB2, rr=RR, wb=WB, ww=WW)
                    for rb_ in range(RB2):
                        nc.vector.tensor_copy(out=PM(xpm)[:, rb_], in_=xr_view[:, rb_])
                    for t in range(NPIX // 512):
                        pt0 = ps.tile([128, 512], f32, tag="pp")
                        nc.tensor.matmul(pt0[0:DIM, :], wcb[...],
                                         attg[:, t * 512:(t + 1) * 512],
                                         start=True, stop=True)
                        nc.vector.scalar_tensor_tensor(
                            out=ot_pm[:, t * 512:(t + 1) * 512],
                            in0=xpm[:, t * 512:(t + 1) * 512],
                            scalar=rsd[...],
                            in1=pt0[0:DIM, :],
                            op0=ALU.mult, op1=ALU.add)
                    for rb_ in range(RB2):
                        nc.vector.tensor_copy(out=RM(ot)[:, rb_], in_=PM(ot_pm)[:, rb_])
                    nc.gpsimd.dma_start(
                        out=outp[b, :, r0:r0 + ROWS, :],
                        in_=ot[...].rearrange("p (r w) -> p r w", r=ROWS, w=W))
    nc.finalize()
    return nc


def kernel(x, shared, freq_emb, noise, gate_w, freq_gate_w, p0, p1, p2,
           q_w, q_dw, kv_w, kv_dw, ln_w, ln_b, po_w, po_b):
    x = np.asarray(x, np.float32)
    shared = np.asarray(shared, np.float32)
    gates = _host_gates(np.asarray(x), np.asarray(freq_emb), np.asarray(noise),
                        np.asarray(gate_w), np.asarray(freq_gate_w))

    # stacked weights
    w_xr = np.zeros((DIM, 128), np.float32)
    w_q = np.zeros((128, 128), np.float32)
    w_k = np.zeros((128, 128), np.float32)
    w_v = np.zeros((128, 128), np.float32)
    w_p1 = np.zeros((DIM, 128), np.float32)
    w_po = np.zeros((128, 128), np.float32)
    t_qa = np.zeros((128, 9), np.float32)
    t_ka = np.zeros((128, 49), np.float32)
    t_va = np.zeros((128, 49), np.float32)
    po_ba = np.zeros((128, 1), np.float32)
    ln_wa = np.zeros((128, 1), np.float32)
    ln_ba = np.zeros((128, 1), np.float32)
    for e in range(E):
        s = e * RANK
        w_xr[:, s:s + RANK] = p0[e].T            # [DIM, RANK]
        w_q[s:s + RANK, s:s + RANK] = q_w[e].T
        w_k[s:s + RANK, s:s + RANK] = kv_w[e][:RANK].T
        w_v[s:s + RANK, s:s + RANK] = kv_w[e][RANK:].T
        w_p1[:, s:s + RANK] = p1[e].T
        w_po[s:s + RANK, s:s + RANK] = po_w[e].T
        t_qa[s:s + RANK] = np.asarray(q_dw)[e, :, 0].reshape(RANK, 9)
        t_ka[s:s + RANK] = np.asarray(kv_dw)[e, :RANK, 0].reshape(RANK, 49)
        t_va[s:s + RANK] = np.asarray(kv_dw)[e, RANK:, 0].reshape(RANK, 49)
        po_ba[s:s + RANK, 0] = po_b[e]
        ln_wa[s:s + RANK, 0] = ln_w[e]
        ln_ba[s:s + RANK, 0] = ln_b[e]

    m_fwd, m_inv = _fft_mats()
    ones_blk = np.zeros((128, 4), np.float32)
    ind = np.zeros((4, 128), np.float32)
    for e in range(E):
        ones_blk[e * RANK:(e + 1) * RANK, e] = 1.0 / RANK
        ind[e, e * RANK:(e + 1) * RANK] = 1.0
    eps_arr = np.full((4, 1), 1e-5, np.float32)

    if "nc" not in _CACHE:
        _CACHE["nc"] = _build()
    nc = _CACHE["nc"]

    com = {
        "w_xr": w_xr.astype(BF), "w_q": w_q.astype(BF), "w_k": w_k.astype(BF),
        "w_v": w_v.astype(BF), "t_q": t_qa, "t_k": t_ka, "t_v": t_va,
        "w_p1": w_p1.astype(BF), "w_po": w_po.astype(BF), "po_b": po_ba,
        "ln_w": ln_wa, "ln_b": ln_ba,
        "m_fr": np.ascontiguousarray(m_fwd[:, 0:40]).astype(BF),
        "m_fi": np.ascontiguousarray(m_fwd[:, 40:80]).astype(BF),
        "m_ir": np.ascontiguousarray(m_inv[0:40]).astype(BF),
        "m_ii": np.ascontiguousarray(m_inv[40:80]).astype(BF),
        "ones_blk": ones_blk.astype(BF), "ind": ind.astype(BF),
        "eps_ap": eps_arr,
    }
    in_maps = []
    p2a = np.asarray(p2, np.float32)
    for c in range(NCORES):
        sl = slice(c * SPC, (c + 1) * SPC)
        wcomb = np.zeros((SPC, 128, DIM), np.float32)
        rs = np.zeros((SPC, DIM, 1), np.float32)
        for i, b in enumerate(range(c * SPC, (c + 1) * SPC)):
            for e in range(E):
                wcomb[i, e * RANK:(e + 1) * RANK, :] = gates[b, e] * p2a[e].T
            rs[i, :, 0] = gates[b].sum()
        m = dict(com)
        m["x"] = np.ascontiguousarray(x[sl])
        m["x_bf"] = np.ascontiguousarray(x[sl]).astype(BF)
        m["shared"] = np.ascontiguousarray(shared[sl]).astype(BF)
        m["w_comb"] = wcomb.astype(BF)
        m["resid"] = rs
        in_maps.append(m)

    res = run_bass_kernel_spmd(nc, in_maps, list(range(NCORES)))
    _CACHE["exec_time_ns"] = res.exec_time_ns
    _CACHE["res"] = res
    outs = [r["out"] for r in res.results]
    return np.concatenate(outs, axis=0).astype(np.float32)

